# Initial kernel scaffold
#
"""Trainium2 Bass kernel for nn_DecoderBlock (B=32, T=512, D=512, H=8, FFN=2048).

Sharding: data-parallel over batch, 4 batch elements per core across 8 cores.
On-chip layout: activations are feature-major (X.T = [d, t]); all weights are
host-pre-transposed so every matmul's stationary operand is a plain contiguous
tile. Softmax is computed without max-subtraction (scores are O(1) scale);
masks arrive as host-precomputed additive -1e10 tiles / per-partition biases.
Matmuls run in float32r (full-speed fp32 mode for free dim >= 256).
"""
import sys

sys.path.insert(0, '/opt/trn_rl_repo')

import numpy as np

D = 512
T = 512
H = 8
DH = 64
FFN = 2048
B = 32
NCORES = 8
NB = B // NCORES  # batch elements per core
P = 128
NDC = D // P     # 4 feature chunks
NHC = FFN // P   # 16 ffn-hidden chunks
NEG = -1.0e10
EPS = 1e-5

_BUILD_CACHE = {}


def build(nb=NB):
    if nb in _BUILD_CACHE:
        return _BUILD_CACHE[nb]

    import concourse.bass as bass  # noqa: F401
    import concourse.tile as tile
    import concourse.mybir as mybir
    from concourse import bacc
    from concourse.alu_op_type import AluOpType

    F32 = mybir.dt.float32
    F32R = mybir.dt.float32r
    AF = mybir.ActivationFunctionType

    def r(ap):
        return ap.bitcast(F32R)

    nc = bacc.Bacc()

    # ---- DRAM I/O ----
    xt = nc.dram_tensor("xt", [nb, D, T], F32, kind="ExternalInput")
    et = nc.dram_tensor("et", [nb, D, T], F32, kind="ExternalInput")
    maskt = nc.dram_tensor("maskt", [nb, T, T], F32, kind="ExternalInput")
    ebias = nc.dram_tensor("ebias", [nb, P, NDC], F32, kind="ExternalInput")
    wname = ["wqt1", "wkt1", "wvt1", "wot1", "wqt2", "wkt2", "wvt2", "wot2"]
    wdr = {n: nc.dram_tensor(n, [D, D], F32, kind="ExternalInput") for n in wname}
    w1t = nc.dram_tensor("w1t", [D, FFN], F32, kind="ExternalInput")
    w2t = nc.dram_tensor("w2t", [FFN, D], F32, kind="ExternalInput")
    b1c = nc.dram_tensor("b1c", [P, NHC], F32, kind="ExternalInput")
    b2c = nc.dram_tensor("b2c", [P, NDC], F32, kind="ExternalInput")
    gbe = {}
    for j in (1, 2, 3):
        gbe[f"g{j}"] = nc.dram_tensor(f"g{j}c", [P, NDC], F32, kind="ExternalInput")
        gbe[f"be{j}"] = nc.dram_tensor(f"be{j}c", [P, NDC], F32, kind="ExternalInput")
    ot = nc.dram_tensor("ot", [nb, D, T], F32, kind="ExternalOutput")

    with tile.TileContext(nc) as tc:
        with tc.tile_pool(name="singles", bufs=1) as singles, \
             tc.tile_pool(name="wa", bufs=8) as wa, \
             tc.tile_pool(name="w1", bufs=4) as w1p, \
             tc.tile_pool(name="w2", bufs=3) as w2p, \
             tc.tile_pool(name="xt", bufs=4) as xtp, \
             tc.tile_pool(name="et", bufs=4) as etp, \
             tc.tile_pool(name="mt", bufs=4) as mtp, \
             tc.tile_pool(name="qt", bufs=4) as qtp, \
             tc.tile_pool(name="kt", bufs=4) as ktp, \
             tc.tile_pool(name="vo", bufs=4) as vop, \
             tc.tile_pool(name="ex", bufs=12) as exp_pool, \
             tc.tile_pool(name="otl", bufs=4) as otp, \
             tc.tile_pool(name="preln", bufs=4) as prelnp, \
             tc.tile_pool(name="postln", bufs=8) as postlnp, \
             tc.tile_pool(name="ht", bufs=4) as htp, \
             tc.tile_pool(name="sm", bufs=4) as smp, \
             tc.tile_pool(name="r65", bufs=2) as r65p, \
             tc.tile_pool(name="stg", bufs=2) as stgp, \
             tc.tile_pool(name="sq", bufs=2) as sqp, \
             tc.tile_pool(name="lnw", bufs=2) as lnwp, \
             tc.tile_pool(name="psA", bufs=3, space="PSUM") as psA, \
             tc.tile_pool(name="psB", bufs=4, space="PSUM") as psB:

            # persistent constants
            ones128 = singles.tile([P, 1], F32, tag="ones128")
            nc.vector.memset(ones128, 1.0)
            ones_row = singles.tile([1, P], F32, tag="ones_row")
            nc.vector.memset(ones_row, 1.0)
            ones65 = singles.tile([65, P], F32, tag="ones65")
            nc.vector.memset(ones65, 1.0)
            tb1 = singles.tile([P, NHC], F32, tag="b1")
            nc.sync.dma_start(out=tb1, in_=b1c.ap())
            tb2 = singles.tile([P, NDC], F32, tag="b2")
            nc.sync.dma_start(out=tb2, in_=b2c.ap())
            tgbe = {}
            for j in (1, 2, 3):
                tgbe[f"g{j}"] = singles.tile([P, NDC], F32, tag=f"g{j}")
                nc.sync.dma_start(out=tgbe[f"g{j}"], in_=gbe[f"g{j}"].ap())
                tgbe[f"be{j}"] = singles.tile([P, NDC], F32, tag=f"be{j}")
                nc.sync.dma_start(out=tgbe[f"be{j}"], in_=gbe[f"be{j}"].ap())

            def load_w_tiles(name, tag):
                """Load a [D, D] pre-transposed weight as 4 [128, D] tiles."""
                tiles = []
                for c in range(NDC):
                    t = wa.tile([P, D], F32, tag="wa")
                    nc.sync.dma_start(out=t, in_=wdr[name].ap()[c * P:(c + 1) * P, :])
                    tiles.append(t)
                return tiles

            def proj_fm(wtiles, src, out_pool, tag):
                """Feature-major projection: out.T[dout,t] = W @ src.T.
                lhsT = wt[kc][:, dc*128:+128], rhs = src[kc]. Returns 4 tiles."""
                outs = []
                for dc in range(NDC):
                    ps = psA.tile([P, T], F32, tag="psA")
                    for kc in range(NDC):
                        nc.tensor.matmul(ps, r(wtiles[kc][:, dc * P:(dc + 1) * P]),
                                         r(src[kc]), start=(kc == 0), stop=(kc == NDC - 1))
                    o = out_pool.tile([P, T], F32, tag=tag)
                    nc.vector.tensor_copy(out=o, in_=ps)
                    outs.append(o)
                return outs

            def proj_vones(wvtiles, src):
                """Token-major V with ones columns: vo[kc] = [128(k), 8*65]."""
                vos = []
                for kc in range(NDC):
                    ps = psA.tile([P, T], F32, tag="psA")
                    for dcd in range(NDC):
                        nc.tensor.matmul(ps, r(src[dcd][:, kc * P:(kc + 1) * P]),
                                         r(wvtiles[dcd]), start=(dcd == 0), stop=(dcd == NDC - 1))
                    vo = vop.tile([P, H * 65], F32, tag="vo")
                    nc.vector.memset(
                        vo.rearrange("p (h c) -> p h c", c=65)[:, :, 64:65], 1.0)
                    for h in range(H):
                        nc.vector.tensor_copy(out=vo[:, h * 65:h * 65 + 64],
                                              in_=ps[:, h * DH:(h + 1) * DH])
                    vos.append(vo)
                return vos

            def attention(qt, kt, vo, is_self, mts=None, ebias_t=None):
                """Multi-head attention; returns 4 OT tiles [128, T] (feature-major)."""
                ot_tiles = [otp.tile([P, T], F32, tag="otl") for _ in range(NDC)]
                e_tiles = {}

                def scores_exp(h):
                    base = (h % 2) * DH
                    cb = h // 2
                    es = []
                    for kc in range(NDC):
                        sp = psA.tile([P, T], F32, tag="psA")
                        nc.tensor.matmul(sp,
                                         r(kt[cb][base:base + DH, kc * P:(kc + 1) * P]),
                                         r(qt[cb][base:base + DH, :]),
                                         start=True, stop=True)
                        e = exp_pool.tile([P, T], F32, tag="ex")
                        if is_self:
                            nc.vector.scalar_tensor_tensor(
                                out=e, in0=sp, scalar=0.125, in1=mts[kc],
                                op0=AluOpType.mult, op1=AluOpType.add)
                            nc.scalar.activation(out=e, in_=e, func=AF.Exp)
                        else:
                            nc.scalar.activation(out=e, in_=sp, func=AF.Exp,
                                                 bias=ebias_t[:, kc:kc + 1], scale=0.125)
                        es.append(e)
                    e_tiles[h] = es

                def pv_norm(h):
                    cb = h // 2
                    es = e_tiles.pop(h)
                    pv = psB.tile([65, T], F32, tag="psB")
                    for kc in range(NDC):
                        nc.tensor.matmul(pv, r(vo[kc][:, h * 65:(h + 1) * 65]),
                                         r(es[kc]), start=(kc == 0), stop=(kc == NDC - 1))
                    r65 = r65p.tile([65, T], F32, tag="r65")
                    nc.vector.reciprocal(out=r65[64:65, :], in_=pv[64:65, :])
                    rb = psA.tile([P, T], F32, tag="psA")
                    nc.tensor.matmul(rb[0:DH, :], r(ones65[64:65, 0:DH]),
                                     r(r65[64:65, :]), start=True, stop=True)
                    if h % 2 == 0:
                        nc.vector.tensor_tensor(out=ot_tiles[cb][0:DH, :],
                                                in0=pv[0:DH, :], in1=rb[0:DH, :],
                                                op=AluOpType.mult)
                    else:
                        stg = stgp.tile([DH, T], F32, tag="stg")
                        nc.vector.tensor_tensor(out=stg, in0=pv[0:DH, :],
                                                in1=rb[0:DH, :], op=AluOpType.mult)
                        nc.sync.dma_start(out=ot_tiles[cb][DH:P, :], in_=stg)

                prev = None
                for h in range(H):
                    scores_exp(h)
                    if prev is not None:
                        pv_norm(prev)
                    prev = h
                pv_norm(prev)
                return ot_tiles

            def out_proj_residual(wtiles, ot_tiles, resid, tag):
                """Y0.T[dc] = Wo @ O.T + resid."""
                outs = []
                for dc in range(NDC):
                    ps = psA.tile([P, T], F32, tag="psA")
                    for ic in range(NDC):
                        nc.tensor.matmul(ps, r(wtiles[ic][:, dc * P:(dc + 1) * P]),
                                         r(ot_tiles[ic]), start=(ic == 0), stop=(ic == NDC - 1))
                    o = prelnp.tile([P, T], F32, tag="preln")
                    nc.vector.scalar_tensor_tensor(out=o, in0=ps, scalar=1.0,
                                                   in1=resid[dc], op0=AluOpType.mult,
                                                   op1=AluOpType.add)
                    outs.append(o)
                return outs

            def layer_norm(src, g, be, tag):
                """Feature-major layernorm over the partition (d) dim."""
                s1 = psB.tile([1, T], F32, tag="psB")
                s2 = psB.tile([1, T], F32, tag="psB")
                for dc in range(NDC):
                    nc.tensor.matmul(s1, r(ones128), r(src[dc]),
                                     start=(dc == 0), stop=(dc == NDC - 1))
                for dc in range(NDC):
                    sq = sqp.tile([P, T], F32, tag="sq")
                    nc.vector.tensor_tensor(out=sq, in0=src[dc], in1=src[dc],
                                            op=AluOpType.mult)
                    nc.tensor.matmul(s2, r(ones128), r(sq),
                                     start=(dc == 0), stop=(dc == NDC - 1))
                s1s = smp.tile([1, T], F32, tag="sm")
                nc.scalar.activation(out=s1s, in_=s1, func=AF.Copy)
                s2s = smp.tile([1, T], F32, tag="sm")
                nc.scalar.activation(out=s2s, in_=s2, func=AF.Copy)
                s1b = psA.tile([P, T], F32, tag="psA")
                nc.tensor.matmul(s1b, r(ones_row), r(s1s), start=True, stop=True)
                s2b = psA.tile([P, T], F32, tag="psA")
                nc.tensor.matmul(s2b, r(ones_row), r(s2s), start=True, stop=True)
                # mean and rstd, full-width [128, T]
                mw = lnwp.tile([P, T], F32, tag="lnw_m")
                nc.vector.tensor_scalar(out=mw, in0=s1b, scalar1=-1.0 / D, scalar2=None,
                                        op0=AluOpType.mult)  # -mu
                msq = lnwp.tile([P, T], F32, tag="lnw_v")
                nc.vector.tensor_tensor(out=msq, in0=mw, in1=mw, op=AluOpType.mult)
                nc.vector.tensor_scalar(out=msq, in0=msq, scalar1=-1.0, scalar2=None,
                                        op0=AluOpType.mult)  # -mu^2
                nc.vector.scalar_tensor_tensor(out=msq, in0=s2b, scalar=1.0 / D,
                                               in1=msq, op0=AluOpType.mult,
                                               op1=AluOpType.add)  # var
                nc.scalar.activation(out=msq, in_=msq, func=AF.Sqrt, bias=EPS)
                nc.vector.reciprocal(out=msq, in_=msq)  # rstd
                outs = []
                for dc in range(NDC):
                    o = postlnp.tile([P, T], F32, tag="postln")
                    nc.vector.tensor_tensor(out=o, in0=src[dc], in1=mw,
                                            op=AluOpType.add)  # x - mu
                    nc.vector.tensor_tensor(out=o, in0=o, in1=msq,
                                            op=AluOpType.mult)  # * rstd
                    nc.vector.tensor_scalar(out=o, in0=o, scalar1=g[:, dc:dc + 1],
                                            scalar2=be[:, dc:dc + 1],
                                            op0=AluOpType.mult, op1=AluOpType.add)
                    outs.append(o)
                return outs

            for b in range(nb):
                # ---- load per-b inputs ----
                xts = []
                for dc in range(NDC):
                    t = xtp.tile([P, T], F32, tag="xt")
                    nc.sync.dma_start(out=t, in_=xt.ap()[b, dc * P:(dc + 1) * P, :])
                    xts.append(t)
                mts = []
                for kc in range(NDC):
                    t = mtp.tile([P, T], F32, tag="mt")
                    nc.sync.dma_start(out=t, in_=maskt.ap()[b, kc * P:(kc + 1) * P, :])
                    mts.append(t)
                ebias_t = smp.tile([P, NDC], F32, tag="sm_eb")
                nc.sync.dma_start(out=ebias_t, in_=ebias.ap()[b])

                # ---- self attention ----
                wq = load_w_tiles("wqt1", "wa")
                qt = proj_fm(wq, xts, qtp, "qt")
                wk = load_w_tiles("wkt1", "wa")
                kt = proj_fm(wk, xts, ktp, "kt")
                wv = load_w_tiles("wvt1", "wa")
                vo = proj_vones(wv, xts)
                ot_t = attention(qt, kt, vo, True, mts=mts)
                wo = load_w_tiles("wot1", "wa")
                y0 = out_proj_residual(wo, ot_t, xts, "y0")
                yt = layer_norm(y0, tgbe["g1"], tgbe["be1"], "yt")

                # ---- cross attention ----
                ets = []
                for dc in range(NDC):
                    t = etp.tile([P, T], F32, tag="et")
                    nc.sync.dma_start(out=t, in_=et.ap()[b, dc * P:(dc + 1) * P, :])
                    ets.append(t)
                wq2 = load_w_tiles("wqt2", "wa")
                qt2 = proj_fm(wq2, yt, qtp, "qt")
                wk2 = load_w_tiles("wkt2", "wa")
                kt2 = proj_fm(wk2, ets, ktp, "kt")
                wv2 = load_w_tiles("wvt2", "wa")
                vo2 = proj_vones(wv2, ets)
                ot2 = attention(qt2, kt2, vo2, False, ebias_t=ebias_t)
                wo2 = load_w_tiles("wot2", "wa")
                z0 = out_proj_residual(wo2, ot2, yt, "z0")
                zt = layer_norm(z0, tgbe["g2"], tgbe["be2"], "zt")

                # ---- FFN ----
                w1tiles = []
                for dc in range(NDC):
                    t = w1p.tile([P, FFN], F32, tag="w1")
                    nc.sync.dma_start(out=t, in_=w1t.ap()[dc * P:(dc + 1) * P, :])
                    w1tiles.append(t)
                fps = [psB.tile([P, T], F32, tag="psB") for _ in range(NDC)]
                h_tiles = {}

                def ffn_h(hc):
                    hp = psA.tile([P, T], F32, tag="psA")
                    for dc in range(NDC):
                        nc.tensor.matmul(hp, r(w1tiles[dc][:, hc * P:(hc + 1) * P]),
                                         r(zt[dc]), start=(dc == 0), stop=(dc == NDC - 1))
                    ht = htp.tile([P, T], F32, tag="ht")
                    nc.vector.tensor_scalar(out=ht, in0=hp, scalar1=tb1[:, hc:hc + 1],
                                            scalar2=None, op0=AluOpType.add)
                    nc.vector.scalar_tensor_tensor(out=ht, in0=ht, scalar=0.01, in1=ht,
                                                   op0=AluOpType.mult, op1=AluOpType.max)
                    h_tiles[hc] = ht

                def ffn_f(hc):
                    ht = h_tiles.pop(hc)
                    w2tile = w2p.tile([P, D], F32, tag="w2")
                    nc.sync.dma_start(out=w2tile, in_=w2t.ap()[hc * P:(hc + 1) * P, :])
                    for dc in range(NDC):
                        nc.tensor.matmul(fps[dc], r(w2tile[:, dc * P:(dc + 1) * P]),
                                         r(h_tiles_keep[hc] if False else ht),
                                         start=(hc == 0), stop=(hc == NHC - 1))

                prevh = None
                for hc in range(NHC):
                    ffn_h(hc)
                    if prevh is not None:
                        ffn_f(prevh)
                    prevh = hc
                ffn_f(prevh)

                out0 = []
                for dc in range(NDC):
                    o = prelnp.tile([P, T], F32, tag="preln")
                    nc.vector.scalar_tensor_tensor(out=o, in0=fps[dc],
                                                   scalar=tb2[:, dc:dc + 1], in1=zt[dc],
                                                   op0=AluOpType.add, op1=AluOpType.add)
                    out0.append(o)
                outt = layer_norm(out0, tgbe["g3"], tgbe["be3"], "outt")
                for dc in range(NDC):
                    nc.sync.dma_start(out=ot.ap()[b, dc * P:(dc + 1) * P, :],
                                      in_=outt[dc])

    nc.compile()
    _BUILD_CACHE[nb] = nc
    return nc


def prep_core_inputs(inputs, nb=NB):
    """Host-side prep: transpose weights/activations, build masks, shard over cores."""
    X = np.asarray(inputs["X"], np.float32)
    E = np.asarray(inputs["enc_outputs"], np.float32)
    dv = np.asarray(inputs["dec_valid_lens"])
    ev = np.asarray(inputs["enc_valid_lens"])
    pos = np.arange(T)

    shared = {
        "w1t": np.ascontiguousarray(np.asarray(inputs["W1"], np.float32).T),
        "w2t": np.ascontiguousarray(np.asarray(inputs["W2"], np.float32).T),
        "b1c": np.ascontiguousarray(np.asarray(inputs["b1"], np.float32).reshape(NHC, P).T),
        "b2c": np.ascontiguousarray(np.asarray(inputs["b2"], np.float32).reshape(NDC, P).T),
    }
    for j in (1, 2, 3):
        shared[f"g{j}c"] = np.ascontiguousarray(
            np.asarray(inputs[f"g{j}"], np.float32).reshape(NDC, P).T)
        shared[f"be{j}c"] = np.ascontiguousarray(
            np.asarray(inputs[f"be{j}"], np.float32).reshape(NDC, P).T)
    for n, src in [("wqt1", "Wq1"), ("wkt1", "Wk1"), ("wvt1", "Wv1"), ("wot1", "Wo1"),
                   ("wqt2", "Wq2"), ("wkt2", "Wk2"), ("wvt2", "Wv2"), ("wot2", "Wo2")]:
        shared[n] = np.ascontiguousarray(np.asarray(inputs[src], np.float32).T)

    in_maps = []
    ncores = X.shape[0] // nb
    for c in range(ncores):
        sl = slice(c * nb, (c + 1) * nb)
        xt = np.ascontiguousarray(X[sl].transpose(0, 2, 1))
        et = np.ascontiguousarray(E[sl].transpose(0, 2, 1))
        # self mask: maskt[b][k, q] = NEG where k >= dec_valid[b, q]
        mk = (pos[None, :, None] >= dv[sl][:, None, :]).astype(np.float32) * NEG
        # cross bias per k: ebias[b, p, kc] for k = kc*128 + p
        eb = (pos[None, :] >= ev[sl][:, None]).astype(np.float32) * NEG
        eb = np.ascontiguousarray(eb.reshape(nb, NDC, P).transpose(0, 2, 1))
        m = {"xt": xt, "et": et, "maskt": np.ascontiguousarray(mk), "ebias": eb}
        m.update(shared)
        in_maps.append(m)
    return in_maps


def kernel(**inputs):
    from concourse import bass_utils

    nc = build(NB)
    in_maps = prep_core_inputs(inputs, NB)
    res = bass_utils.run_bass_kernel_spmd(nc, in_maps, core_ids=list(range(NCORES)))
    outs = [r["ot"].transpose(0, 2, 1) for r in res.results]  # [nb, T, D] each
    return np.ascontiguousarray(np.concatenate(outs, axis=0).astype(np.float32))


# revision 17
# speedup vs baseline: 1.0021x; 1.0021x over previous
"""Trainium2 Bass kernel for nn_DecoderBlock (B=32, T=512, D=512, H=8, FFN=2048).

Sharding: data-parallel over batch, 4 batch elements per core across 8 cores.
On-chip layout: activations are feature-major (X.T = [d, t]); all weights are
host-pre-transposed so every matmul's stationary operand is a plain contiguous
tile. Softmax is computed without max-subtraction (scores are O(1) scale);
masks arrive as host-precomputed additive -1e10 tiles / per-partition biases.
Matmuls run in float32r (full-speed fp32 mode for free dim >= 256).
"""
import sys

sys.path.insert(0, '/opt/trn_rl_repo')

import numpy as np

D = 512
T = 512
H = 8
DH = 64
FFN = 2048
B = 32
NCORES = 8
NB = B // NCORES  # batch elements per core
P = 128
NDC = D // P     # 4 feature chunks
NHC = FFN // P   # 16 ffn-hidden chunks
NEG = -1.0e10
EPS = 1e-5

_BUILD_CACHE = {}


def build(nb=NB, reps=1):
    key = (nb, reps)
    if key in _BUILD_CACHE:
        return _BUILD_CACHE[key]

    import concourse.bass as bass  # noqa: F401
    import concourse.tile as tile
    import concourse.mybir as mybir
    from concourse import bacc
    from concourse.alu_op_type import AluOpType

    F32 = mybir.dt.float32
    F32R = mybir.dt.float32r
    AF = mybir.ActivationFunctionType

    def r(ap):
        return ap.bitcast(F32R)

    nc = bacc.Bacc()

    # ---- DRAM I/O ----
    xt = nc.dram_tensor("xt", [nb, D, T], F32R, kind="ExternalInput")
    et = nc.dram_tensor("et", [nb, D, T], F32R, kind="ExternalInput")
    maskt = nc.dram_tensor("maskt", [nb, T, T], F32, kind="ExternalInput")
    ebias = nc.dram_tensor("ebias", [nb, P, NDC], F32, kind="ExternalInput")
    wname = ["wqt1", "wkt1", "wvt1", "wot1", "wqt2", "wkt2", "wvt2", "wot2"]
    wdr = {n: nc.dram_tensor(n, [D, D], F32R, kind="ExternalInput") for n in wname}
    w1t = nc.dram_tensor("w1t", [D, FFN], F32R, kind="ExternalInput")
    w2t = nc.dram_tensor("w2t", [FFN, D], F32R, kind="ExternalInput")
    b1c = nc.dram_tensor("b1c", [P, NHC], F32, kind="ExternalInput")
    b2c = nc.dram_tensor("b2c", [P, NDC], F32, kind="ExternalInput")
    gbe = {}
    for j in (1, 2, 3):
        gbe[f"g{j}"] = nc.dram_tensor(f"g{j}c", [P, NDC], F32, kind="ExternalInput")
        gbe[f"be{j}"] = nc.dram_tensor(f"be{j}c", [P, NDC], F32, kind="ExternalInput")
    ot = nc.dram_tensor("ot", [nb, D, T], F32, kind="ExternalOutput")

    from contextlib import ExitStack

    with tile.TileContext(nc) as tc:
        with ExitStack() as ctx:
            ctx.enter_context(nc.allow_low_precision(
                reason="fp32r is fp32-width; rounding only trims low mantissa bits"))
            singles = ctx.enter_context(tc.tile_pool(name="singles", bufs=1))
            wa = ctx.enter_context(tc.tile_pool(name="wa", bufs=8))
            w1p = ctx.enter_context(tc.tile_pool(name="w1", bufs=4))
            w2p = ctx.enter_context(tc.tile_pool(name="w2", bufs=3))
            xtp = ctx.enter_context(tc.tile_pool(name="xt", bufs=4))
            etp = ctx.enter_context(tc.tile_pool(name="et", bufs=4))
            mtp = ctx.enter_context(tc.tile_pool(name="mt", bufs=4))
            qtp = ctx.enter_context(tc.tile_pool(name="qt", bufs=4))
            ktp = ctx.enter_context(tc.tile_pool(name="kt", bufs=4))
            vop = ctx.enter_context(tc.tile_pool(name="vo", bufs=4))
            exp_pool = ctx.enter_context(tc.tile_pool(name="ex", bufs=12))
            otp = ctx.enter_context(tc.tile_pool(name="otl", bufs=4))
            prelnp = ctx.enter_context(tc.tile_pool(name="preln", bufs=4))
            postlnp = ctx.enter_context(tc.tile_pool(name="postln", bufs=8))
            htp = ctx.enter_context(tc.tile_pool(name="ht", bufs=4))
            smp = ctx.enter_context(tc.tile_pool(name="sm", bufs=4))
            r65p = ctx.enter_context(tc.tile_pool(name="r65", bufs=2))
            stgp = ctx.enter_context(tc.tile_pool(name="stg", bufs=2))
            sqp = ctx.enter_context(tc.tile_pool(name="sq", bufs=2))
            lnwp = ctx.enter_context(tc.tile_pool(name="lnw", bufs=2))
            psA = ctx.enter_context(tc.tile_pool(name="psA", bufs=3, space="PSUM"))
            psB = ctx.enter_context(tc.tile_pool(name="psB", bufs=4, space="PSUM"))

            # persistent constants (memset can't write fp32r; stage + rounded copy)
            ones_stage = singles.tile([P, P], F32, tag="ones_stage")
            nc.vector.memset(ones_stage, 1.0)
            ones128 = singles.tile([P, 1], F32, tag="ones128")
            nc.vector.tensor_copy(out=r(ones128), in_=ones_stage[:, 0:1])
            ones_row = singles.tile([1, P], F32, tag="ones_row")
            nc.vector.tensor_copy(out=r(ones_row), in_=ones_stage[0:1, :])
            ones65 = singles.tile([65, P], F32, tag="ones65")
            nc.vector.tensor_copy(out=r(ones65), in_=ones_stage[0:65, :])
            eps_t = singles.tile([P, 1], F32, tag="eps")
            nc.vector.memset(eps_t, EPS)
            tb1 = singles.tile([P, NHC], F32, tag="b1")
            nc.sync.dma_start(out=tb1, in_=b1c.ap())
            tb2 = singles.tile([P, NDC], F32, tag="b2")
            nc.sync.dma_start(out=tb2, in_=b2c.ap())
            tgbe = {}
            for j in (1, 2, 3):
                tgbe[f"g{j}"] = singles.tile([P, NDC], F32, tag=f"g{j}", name=f"g{j}t")
                nc.sync.dma_start(out=tgbe[f"g{j}"], in_=gbe[f"g{j}"].ap())
                tgbe[f"be{j}"] = singles.tile([P, NDC], F32, tag=f"be{j}", name=f"be{j}t")
                nc.sync.dma_start(out=tgbe[f"be{j}"], in_=gbe[f"be{j}"].ap())

            def load_w_tiles(name, tag):
                """Load a [D, D] pre-transposed weight as 4 [128, D] tiles."""
                tiles = []
                for c in range(NDC):
                    t = wa.tile([P, D], F32, tag="wa")
                    nc.sync.dma_start(out=r(t), in_=wdr[name].ap()[c * P:(c + 1) * P, :])
                    tiles.append(t)
                return tiles

            def proj_fm(wtiles, src, out_pool, tag):
                """Feature-major projection: out.T[dout,t] = W @ src.T.
                lhsT = wt[kc][:, dc*128:+128], rhs = src[kc]. Returns 4 tiles."""
                outs = []
                for dc in range(NDC):
                    ps = psA.tile([P, T], F32, tag="psA")
                    for kc in range(NDC):
                        nc.tensor.matmul(ps, r(wtiles[kc][:, dc * P:(dc + 1) * P]),
                                         r(src[kc]), start=(kc == 0), stop=(kc == NDC - 1))
                    o = out_pool.tile([P, T], F32, tag=tag)
                    nc.vector.tensor_copy(out=r(o), in_=ps)
                    outs.append(o)
                return outs

            def proj_vones(wvtiles, src):
                """Token-major V with ones columns: vo[kc] = [128(k), 8*65]."""
                vos = []
                for kc in range(NDC):
                    ps = psA.tile([P, T], F32, tag="psA")
                    for dcd in range(NDC):
                        nc.tensor.matmul(ps, r(src[dcd][:, kc * P:(kc + 1) * P]),
                                         r(wvtiles[dcd]), start=(dcd == 0), stop=(dcd == NDC - 1))
                    vo = vop.tile([P, H * 65], F32, tag="vo")
                    nc.vector.tensor_copy(
                        out=r(vo.rearrange("p (h c) -> p h c", c=65)[:, :, 64:65]),
                        in_=ones_stage[:, 0:H].rearrange("p (h c) -> p h c", c=1))
                    for h in range(H):
                        nc.vector.tensor_copy(out=r(vo[:, h * 65:h * 65 + 64]),
                                              in_=ps[:, h * DH:(h + 1) * DH])
                    vos.append(vo)
                return vos

            def attention(qt, kt, vo, is_self, mts=None, ebias_t=None):
                """Multi-head attention; returns 4 OT tiles [128, T] (feature-major)."""
                ot_tiles = [otp.tile([P, T], F32, tag="otl", name=f"otl{i}") for i in range(NDC)]
                e_tiles = {}

                def scores_exp(h):
                    base = (h % 2) * DH
                    cb = h // 2
                    es = []
                    for kc in range(NDC):
                        sp = psA.tile([P, T], F32, tag="psA")
                        nc.tensor.matmul(sp,
                                         r(kt[cb][base:base + DH, kc * P:(kc + 1) * P]),
                                         r(qt[cb][base:base + DH, :]),
                                         start=True, stop=True)
                        e = exp_pool.tile([P, T], F32, tag="ex")
                        if is_self:
                            nc.vector.scalar_tensor_tensor(
                                out=r(e), in0=sp, scalar=0.125, in1=mts[kc],
                                op0=AluOpType.mult, op1=AluOpType.add)
                            nc.scalar.activation(out=r(e), in_=e, func=AF.Exp)
                        else:
                            nc.scalar.activation(out=r(e), in_=sp, func=AF.Exp,
                                                 bias=ebias_t[:, kc:kc + 1], scale=0.125)
                        es.append(e)
                    e_tiles[h] = es

                def pv_norm(h):
                    cb = h // 2
                    es = e_tiles.pop(h)
                    pv = psB.tile([65, T], F32, tag="psB")
                    for kc in range(NDC):
                        nc.tensor.matmul(pv, r(vo[kc][:, h * 65:(h + 1) * 65]),
                                         r(es[kc]), start=(kc == 0), stop=(kc == NDC - 1))
                    r65 = r65p.tile([65, T], F32, tag="r65")
                    nc.vector.reciprocal(out=r(r65[64:65, :]), in_=pv[64:65, :])
                    rb = psA.tile([P, T], F32, tag="psA")
                    nc.tensor.matmul(rb[0:DH, :], r(ones65[64:65, 0:DH]),
                                     r(r65[64:65, :]), start=True, stop=True)
                    rbs = stgp.tile([DH, T], F32, tag="rbs")
                    nc.vector.tensor_copy(out=rbs, in_=rb[0:DH, :])
                    if h % 2 == 0:
                        nc.vector.tensor_tensor(out=r(ot_tiles[cb][0:DH, :]),
                                                in0=pv[0:DH, :], in1=rbs,
                                                op=AluOpType.mult)
                    else:
                        stg = stgp.tile([DH, T], F32, tag="stg")
                        nc.vector.tensor_tensor(out=r(stg), in0=pv[0:DH, :],
                                                in1=rbs, op=AluOpType.mult)
                        nc.sync.dma_start(out=r(ot_tiles[cb][DH:P, :]), in_=r(stg))

                prev = None
                for h in range(H):
                    scores_exp(h)
                    if prev is not None:
                        pv_norm(prev)
                    prev = h
                pv_norm(prev)
                return ot_tiles

            def out_proj_residual(wtiles, ot_tiles, resid, tag):
                """Y0.T[dc] = Wo @ O.T + resid."""
                outs = []
                for dc in range(NDC):
                    ps = psA.tile([P, T], F32, tag="psA")
                    for ic in range(NDC):
                        nc.tensor.matmul(ps, r(wtiles[ic][:, dc * P:(dc + 1) * P]),
                                         r(ot_tiles[ic]), start=(ic == 0), stop=(ic == NDC - 1))
                    o = prelnp.tile([P, T], F32, tag="preln")
                    nc.vector.scalar_tensor_tensor(out=r(o), in0=ps, scalar=1.0,
                                                   in1=resid[dc], op0=AluOpType.mult,
                                                   op1=AluOpType.add)
                    outs.append(o)
                return outs

            def layer_norm(src, g, be, tag, round_out=True):
                """Feature-major layernorm over the partition (d) dim."""
                s1 = psB.tile([1, T], F32, tag="psB")
                s2 = psB.tile([1, T], F32, tag="psB")
                for dc in range(NDC):
                    nc.tensor.matmul(s1, r(ones128), r(src[dc]),
                                     start=(dc == 0), stop=(dc == NDC - 1))
                for dc in range(NDC):
                    sq = sqp.tile([P, T], F32, tag="sq")
                    nc.vector.tensor_tensor(out=r(sq), in0=src[dc], in1=src[dc],
                                            op=AluOpType.mult)
                    nc.tensor.matmul(s2, r(ones128), r(sq),
                                     start=(dc == 0), stop=(dc == NDC - 1))
                s1s = smp.tile([1, T], F32, tag="sm")
                nc.scalar.activation(out=r(s1s), in_=s1, func=AF.Copy)
                s2s = smp.tile([1, T], F32, tag="sm")
                nc.scalar.activation(out=r(s2s), in_=s2, func=AF.Copy)
                s1b = psA.tile([P, T], F32, tag="psA")
                nc.tensor.matmul(s1b, r(ones_row), r(s1s), start=True, stop=True)
                s2b = psA.tile([P, T], F32, tag="psA")
                nc.tensor.matmul(s2b, r(ones_row), r(s2s), start=True, stop=True)
                # mean and rstd, full-width [128, T]
                mw = lnwp.tile([P, T], F32, tag="lnw_m")
                nc.vector.tensor_scalar(out=mw, in0=s1b, scalar1=-1.0 / D, scalar2=None,
                                        op0=AluOpType.mult)  # -mu
                msq = lnwp.tile([P, T], F32, tag="lnw_v")
                nc.vector.tensor_tensor(out=msq, in0=mw, in1=mw, op=AluOpType.mult)
                nc.vector.tensor_scalar(out=msq, in0=msq, scalar1=-1.0, scalar2=None,
                                        op0=AluOpType.mult)  # -mu^2
                nc.vector.scalar_tensor_tensor(out=msq, in0=s2b, scalar=1.0 / D,
                                               in1=msq, op0=AluOpType.mult,
                                               op1=AluOpType.add)  # var
                nc.scalar.activation(out=msq, in_=msq, func=AF.Sqrt, bias=eps_t[:, 0:1])
                nc.vector.reciprocal(out=msq, in_=msq)  # rstd
                outs = []
                for dc in range(NDC):
                    o = postlnp.tile([P, T], F32, tag="postln")
                    ow = r(o) if round_out else o
                    nc.vector.tensor_tensor(out=ow, in0=src[dc], in1=mw,
                                            op=AluOpType.add)  # x - mu
                    nc.vector.tensor_tensor(out=ow, in0=o, in1=msq,
                                            op=AluOpType.mult)  # * rstd
                    nc.vector.tensor_scalar(out=(r(o) if round_out else o),
                                            in0=o, scalar1=g[:, dc:dc + 1],
                                            scalar2=be[:, dc:dc + 1],
                                            op0=AluOpType.mult, op1=AluOpType.add)
                    outs.append(o)
                return outs

            for b in [bb for _ in range(reps) for bb in range(nb)]:
                # ---- load per-b inputs ----
                xts = []
                for dc in range(NDC):
                    t = xtp.tile([P, T], F32, tag="xt")
                    nc.sync.dma_start(out=r(t), in_=xt.ap()[b, dc * P:(dc + 1) * P, :])
                    xts.append(t)
                mts = []
                for kc in range(NDC):
                    t = mtp.tile([P, T], F32, tag="mt")
                    nc.sync.dma_start(out=t, in_=maskt.ap()[b, kc * P:(kc + 1) * P, :])
                    mts.append(t)
                ebias_t = smp.tile([P, NDC], F32, tag="sm_eb")
                nc.sync.dma_start(out=ebias_t, in_=ebias.ap()[b])

                # ---- self attention ----
                wq = load_w_tiles("wqt1", "wa")
                qt = proj_fm(wq, xts, qtp, "qt")
                wk = load_w_tiles("wkt1", "wa")
                kt = proj_fm(wk, xts, ktp, "kt")
                wv = load_w_tiles("wvt1", "wa")
                vo = proj_vones(wv, xts)
                ot_t = attention(qt, kt, vo, True, mts=mts)
                wo = load_w_tiles("wot1", "wa")
                y0 = out_proj_residual(wo, ot_t, xts, "y0")
                yt = layer_norm(y0, tgbe["g1"], tgbe["be1"], "yt")

                # ---- cross attention ----
                ets = []
                for dc in range(NDC):
                    t = etp.tile([P, T], F32, tag="et")
                    nc.sync.dma_start(out=r(t), in_=et.ap()[b, dc * P:(dc + 1) * P, :])
                    ets.append(t)
                wq2 = load_w_tiles("wqt2", "wa")
                qt2 = proj_fm(wq2, yt, qtp, "qt")
                wk2 = load_w_tiles("wkt2", "wa")
                kt2 = proj_fm(wk2, ets, ktp, "kt")
                wv2 = load_w_tiles("wvt2", "wa")
                vo2 = proj_vones(wv2, ets)
                ot2 = attention(qt2, kt2, vo2, False, ebias_t=ebias_t)
                wo2 = load_w_tiles("wot2", "wa")
                z0 = out_proj_residual(wo2, ot2, yt, "z0")
                zt = layer_norm(z0, tgbe["g2"], tgbe["be2"], "zt")

                # ---- FFN ----
                w1tiles = []
                for dc in range(NDC):
                    t = w1p.tile([P, FFN], F32, tag="w1")
                    nc.sync.dma_start(out=r(t), in_=w1t.ap()[dc * P:(dc + 1) * P, :])
                    w1tiles.append(t)
                fps = [psB.tile([P, T], F32, tag="psB", name=f"fps{i}") for i in range(NDC)]
                h_tiles = {}

                def ffn_h(hc):
                    hp = psA.tile([P, T], F32, tag="psA")
                    for dc in range(NDC):
                        nc.tensor.matmul(hp, r(w1tiles[dc][:, hc * P:(hc + 1) * P]),
                                         r(zt[dc]), start=(dc == 0), stop=(dc == NDC - 1))
                    ht = htp.tile([P, T], F32, tag="ht")
                    nc.vector.tensor_scalar(out=r(ht), in0=hp, scalar1=tb1[:, hc:hc + 1],
                                            scalar2=None, op0=AluOpType.add)
                    nc.vector.scalar_tensor_tensor(out=r(ht), in0=ht, scalar=0.01, in1=ht,
                                                   op0=AluOpType.mult, op1=AluOpType.max)
                    h_tiles[hc] = ht

                def ffn_f(hc):
                    ht = h_tiles.pop(hc)
                    w2tile = w2p.tile([P, D], F32, tag="w2")
                    nc.sync.dma_start(out=r(w2tile), in_=w2t.ap()[hc * P:(hc + 1) * P, :])
                    for dc in range(NDC):
                        nc.tensor.matmul(fps[dc], r(w2tile[:, dc * P:(dc + 1) * P]),
                                         r(ht), start=(hc == 0), stop=(hc == NHC - 1))

                prevh = None
                for hc in range(NHC):
                    ffn_h(hc)
                    if prevh is not None:
                        ffn_f(prevh)
                    prevh = hc
                ffn_f(prevh)

                out0 = []
                for dc in range(NDC):
                    o = prelnp.tile([P, T], F32, tag="preln")
                    nc.vector.scalar_tensor_tensor(out=r(o), in0=fps[dc],
                                                   scalar=tb2[:, dc:dc + 1], in1=zt[dc],
                                                   op0=AluOpType.add, op1=AluOpType.add)
                    out0.append(o)
                outt = layer_norm(out0, tgbe["g3"], tgbe["be3"], "outt", round_out=False)
                for dc in range(NDC):
                    nc.sync.dma_start(out=ot.ap()[b, dc * P:(dc + 1) * P, :],
                                      in_=outt[dc])

    nc.compile()
    _BUILD_CACHE[key] = nc
    return nc


def prep_core_inputs(inputs, nb=NB):
    """Host-side prep: transpose weights/activations, build masks, shard over cores."""
    X = np.asarray(inputs["X"], np.float32)
    E = np.asarray(inputs["enc_outputs"], np.float32)
    dv = np.asarray(inputs["dec_valid_lens"])
    ev = np.asarray(inputs["enc_valid_lens"])
    pos = np.arange(T)

    shared = {
        "w1t": np.ascontiguousarray(np.asarray(inputs["W1"], np.float32).T),
        "w2t": np.ascontiguousarray(np.asarray(inputs["W2"], np.float32).T),
        "b1c": np.ascontiguousarray(np.asarray(inputs["b1"], np.float32).reshape(NHC, P).T),
        "b2c": np.ascontiguousarray(np.asarray(inputs["b2"], np.float32).reshape(NDC, P).T),
    }
    for j in (1, 2, 3):
        shared[f"g{j}c"] = np.ascontiguousarray(
            np.asarray(inputs[f"g{j}"], np.float32).reshape(NDC, P).T)
        shared[f"be{j}c"] = np.ascontiguousarray(
            np.asarray(inputs[f"be{j}"], np.float32).reshape(NDC, P).T)
    for n, src in [("wqt1", "Wq1"), ("wkt1", "Wk1"), ("wvt1", "Wv1"), ("wot1", "Wo1"),
                   ("wqt2", "Wq2"), ("wkt2", "Wk2"), ("wvt2", "Wv2"), ("wot2", "Wo2")]:
        shared[n] = np.ascontiguousarray(np.asarray(inputs[src], np.float32).T)

    in_maps = []
    ncores = X.shape[0] // nb
    for c in range(ncores):
        sl = slice(c * nb, (c + 1) * nb)
        xt = np.ascontiguousarray(X[sl].transpose(0, 2, 1))
        et = np.ascontiguousarray(E[sl].transpose(0, 2, 1))
        # self mask: maskt[b][k, q] = NEG where k >= dec_valid[b, q]
        mk = (pos[None, :, None] >= dv[sl][:, None, :]).astype(np.float32) * NEG
        # cross bias per k: ebias[b, p, kc] for k = kc*128 + p
        eb = (pos[None, :] >= ev[sl][:, None]).astype(np.float32) * NEG
        eb = np.ascontiguousarray(eb.reshape(nb, NDC, P).transpose(0, 2, 1))
        m = {"xt": xt, "et": et, "maskt": np.ascontiguousarray(mk), "ebias": eb}
        m.update(shared)
        in_maps.append(m)
    return in_maps


def kernel(**inputs):
    from concourse import bass_utils

    nc = build(NB)
    in_maps = prep_core_inputs(inputs, NB)
    res = bass_utils.run_bass_kernel_spmd(nc, in_maps, core_ids=list(range(NCORES)))
    outs = [r["ot"].transpose(0, 2, 1) for r in res.results]  # [nb, T, D] each
    return np.ascontiguousarray(np.concatenate(outs, axis=0).astype(np.float32))


# revision 20
# speedup vs baseline: 1.0296x; 1.0275x over previous
"""Trainium2 Bass kernel for nn_DecoderBlock (B=32, T=512, D=512, H=8, FFN=2048).

Sharding: data-parallel over batch, 4 batch elements per core across 8 cores.
On-chip layout: activations are feature-major (X.T = [d, t]); all weights are
host-pre-transposed so every matmul's stationary operand is a plain contiguous
tile. Softmax is computed without max-subtraction (scores are O(1) scale);
masks arrive as host-precomputed additive -1e10 tiles / per-partition biases.
Matmuls run in float32r (full-speed fp32 mode for free dim >= 256).
"""
import sys

sys.path.insert(0, '/opt/trn_rl_repo')

import numpy as np

D = 512
T = 512
H = 8
DH = 64
FFN = 2048
B = 32
NCORES = 8
NB = B // NCORES  # batch elements per core
P = 128
NDC = D // P     # 4 feature chunks
NHC = FFN // P   # 16 ffn-hidden chunks
NEG = -1.0e10
EPS = 1e-5

_BUILD_CACHE = {}


def build(nb=NB, reps=1, loop_n=0):
    key = (nb, reps, loop_n)
    if key in _BUILD_CACHE:
        return _BUILD_CACHE[key]

    import concourse.bass as bass  # noqa: F401
    import concourse.tile as tile
    import concourse.mybir as mybir
    from concourse import bacc
    from concourse.alu_op_type import AluOpType

    F32 = mybir.dt.float32
    F32R = mybir.dt.float32r
    AF = mybir.ActivationFunctionType

    def r(ap):
        return ap.bitcast(F32R)

    nc = bacc.Bacc()

    # ---- DRAM I/O ----
    xt = nc.dram_tensor("xt", [nb, D, T], F32R, kind="ExternalInput")
    et = nc.dram_tensor("et", [nb, D, T], F32R, kind="ExternalInput")
    maskt = nc.dram_tensor("maskt", [nb, T, T], F32, kind="ExternalInput")
    ebias = nc.dram_tensor("ebias", [nb, P, NDC], F32, kind="ExternalInput")
    wname = ["wqt1", "wkt1", "wvt1", "wot1", "wqt2", "wkt2", "wvt2", "wot2"]
    wdr = {n: nc.dram_tensor(n, [D, D], F32R, kind="ExternalInput") for n in wname}
    w1t = nc.dram_tensor("w1t", [D, FFN], F32R, kind="ExternalInput")
    w2t = nc.dram_tensor("w2t", [FFN, D], F32R, kind="ExternalInput")
    b1c = nc.dram_tensor("b1c", [P, NHC], F32, kind="ExternalInput")
    b2c = nc.dram_tensor("b2c", [P, NDC], F32, kind="ExternalInput")
    gbe = {}
    for j in (1, 2, 3):
        gbe[f"g{j}"] = nc.dram_tensor(f"g{j}c", [P, NDC], F32, kind="ExternalInput")
        gbe[f"be{j}"] = nc.dram_tensor(f"be{j}c", [P, NDC], F32, kind="ExternalInput")
    ot = nc.dram_tensor("ot", [nb, D, T], F32, kind="ExternalOutput")

    from contextlib import ExitStack

    with tile.TileContext(nc) as tc:
        with ExitStack() as ctx:
            ctx.enter_context(nc.allow_low_precision(
                reason="fp32r is fp32-width; rounding only trims low mantissa bits"))
            singles = ctx.enter_context(tc.tile_pool(name="singles", bufs=1))
            wa = ctx.enter_context(tc.tile_pool(name="wa", bufs=8))
            w1p = ctx.enter_context(tc.tile_pool(name="w1", bufs=4))
            w2p = ctx.enter_context(tc.tile_pool(name="w2", bufs=3))
            xtp = ctx.enter_context(tc.tile_pool(name="xt", bufs=4))
            etp = ctx.enter_context(tc.tile_pool(name="et", bufs=4))
            mtp = ctx.enter_context(tc.tile_pool(name="mt", bufs=4))
            qtp = ctx.enter_context(tc.tile_pool(name="qt", bufs=4))
            ktp = ctx.enter_context(tc.tile_pool(name="kt", bufs=4))
            vop = ctx.enter_context(tc.tile_pool(name="vo", bufs=4))
            exp_pool = ctx.enter_context(tc.tile_pool(name="ex", bufs=12))
            otp = ctx.enter_context(tc.tile_pool(name="otl", bufs=4))
            prelnp = ctx.enter_context(tc.tile_pool(name="preln", bufs=4))
            postlnp = ctx.enter_context(tc.tile_pool(name="postln", bufs=8))
            htp = ctx.enter_context(tc.tile_pool(name="ht", bufs=4))
            smp = ctx.enter_context(tc.tile_pool(name="sm", bufs=4))
            r65p = ctx.enter_context(tc.tile_pool(name="r65", bufs=2))
            stgp = ctx.enter_context(tc.tile_pool(name="stg", bufs=2))
            sqp = ctx.enter_context(tc.tile_pool(name="sq", bufs=2))
            lnwp = ctx.enter_context(tc.tile_pool(name="lnw", bufs=2))
            psA = ctx.enter_context(tc.tile_pool(name="psA", bufs=3, space="PSUM"))
            psB = ctx.enter_context(tc.tile_pool(name="psB", bufs=4, space="PSUM"))

            # persistent constants (memset can't write fp32r; stage + rounded copy)
            ones_stage = singles.tile([P, P], F32, tag="ones_stage")
            nc.vector.memset(ones_stage, 1.0)
            ones128 = singles.tile([P, 1], F32, tag="ones128")
            nc.vector.tensor_copy(out=r(ones128), in_=ones_stage[:, 0:1])
            ones_row = singles.tile([1, P], F32, tag="ones_row")
            nc.vector.tensor_copy(out=r(ones_row), in_=ones_stage[0:1, :])
            ones65 = singles.tile([65, P], F32, tag="ones65")
            nc.vector.tensor_copy(out=r(ones65), in_=ones_stage[0:65, :])
            eps_t = singles.tile([P, 1], F32, tag="eps")
            nc.vector.memset(eps_t, EPS)
            tb1 = singles.tile([P, NHC], F32, tag="b1")
            nc.sync.dma_start(out=tb1, in_=b1c.ap())
            tb2 = singles.tile([P, NDC], F32, tag="b2")
            nc.sync.dma_start(out=tb2, in_=b2c.ap())
            tgbe = {}
            for j in (1, 2, 3):
                tgbe[f"g{j}"] = singles.tile([P, NDC], F32, tag=f"g{j}", name=f"g{j}t")
                nc.sync.dma_start(out=tgbe[f"g{j}"], in_=gbe[f"g{j}"].ap())
                tgbe[f"be{j}"] = singles.tile([P, NDC], F32, tag=f"be{j}", name=f"be{j}t")
                nc.sync.dma_start(out=tgbe[f"be{j}"], in_=gbe[f"be{j}"].ap())

            def load_w_tiles(name, tag):
                """Load a [D, D] pre-transposed weight as 4 [128, D] tiles."""
                tiles = []
                for c in range(NDC):
                    t = wa.tile([P, D], F32, tag="wa")
                    nc.sync.dma_start(out=r(t), in_=wdr[name].ap()[c * P:(c + 1) * P, :])
                    tiles.append(t)
                return tiles

            def proj_fm(wtiles, src, out_pool, tag):
                """Feature-major projection: out.T[dout,t] = W @ src.T.
                lhsT = wt[kc][:, dc*128:+128], rhs = src[kc]. Returns 4 tiles."""
                outs = []
                for dc in range(NDC):
                    ps = psA.tile([P, T], F32, tag="psA")
                    for kc in range(NDC):
                        nc.tensor.matmul(ps, r(wtiles[kc][:, dc * P:(dc + 1) * P]),
                                         r(src[kc]), start=(kc == 0), stop=(kc == NDC - 1))
                    o = out_pool.tile([P, T], F32, tag=tag)
                    nc.vector.tensor_copy(out=r(o), in_=ps)
                    outs.append(o)
                return outs

            def proj_vones(wvtiles, src):
                """Token-major V with ones columns: vo[kc] = [128(k), 8*65]."""
                vos = []
                for kc in range(NDC):
                    ps = psA.tile([P, T], F32, tag="psA")
                    for dcd in range(NDC):
                        nc.tensor.matmul(ps, r(src[dcd][:, kc * P:(kc + 1) * P]),
                                         r(wvtiles[dcd]), start=(dcd == 0), stop=(dcd == NDC - 1))
                    vo = vop.tile([P, H * 65], F32, tag="vo")
                    nc.vector.tensor_copy(
                        out=r(vo.rearrange("p (h c) -> p h c", c=65)[:, :, 64:65]),
                        in_=ones_stage[:, 0:H].rearrange("p (h c) -> p h c", c=1))
                    for h in range(H):
                        nc.vector.tensor_copy(out=r(vo[:, h * 65:h * 65 + 64]),
                                              in_=ps[:, h * DH:(h + 1) * DH])
                    vos.append(vo)
                return vos

            def attention(qt, kt, vo, is_self, mts=None, ebias_t=None):
                """Multi-head attention; returns 4 OT tiles [128, T] (feature-major)."""
                ot_tiles = [otp.tile([P, T], F32, tag="otl", name=f"otl{i}") for i in range(NDC)]
                e_tiles = {}

                def scores_exp(h):
                    base = (h % 2) * DH
                    cb = h // 2
                    es = []
                    for kc in range(NDC):
                        sp = psA.tile([P, T], F32, tag="psA")
                        nc.tensor.matmul(sp,
                                         r(kt[cb][base:base + DH, kc * P:(kc + 1) * P]),
                                         r(qt[cb][base:base + DH, :]),
                                         start=True, stop=True)
                        e = exp_pool.tile([P, T], F32, tag="ex")
                        if is_self:
                            nc.vector.scalar_tensor_tensor(
                                out=r(e), in0=sp, scalar=0.125, in1=mts[kc],
                                op0=AluOpType.mult, op1=AluOpType.add)
                            nc.scalar.activation(out=r(e), in_=e, func=AF.Exp)
                        else:
                            nc.scalar.activation(out=r(e), in_=sp, func=AF.Exp,
                                                 bias=ebias_t[:, kc:kc + 1], scale=0.125)
                        es.append(e)
                    e_tiles[h] = es

                def pv_norm(h):
                    cb = h // 2
                    es = e_tiles.pop(h)
                    pv = psB.tile([65, T], F32, tag="psB")
                    for kc in range(NDC):
                        nc.tensor.matmul(pv, r(vo[kc][:, h * 65:(h + 1) * 65]),
                                         r(es[kc]), start=(kc == 0), stop=(kc == NDC - 1))
                    r65 = r65p.tile([65, T], F32, tag="r65")
                    nc.vector.reciprocal(out=r(r65[64:65, :]), in_=pv[64:65, :])
                    rb = psA.tile([P, T], F32, tag="psA")
                    nc.tensor.matmul(rb[0:DH, :], r(ones65[64:65, 0:DH]),
                                     r(r65[64:65, :]), start=True, stop=True)
                    rbs = stgp.tile([DH, T], F32, tag="rbs")
                    nc.vector.tensor_copy(out=rbs, in_=rb[0:DH, :])
                    if h % 2 == 0:
                        nc.vector.tensor_tensor(out=r(ot_tiles[cb][0:DH, :]),
                                                in0=pv[0:DH, :], in1=rbs,
                                                op=AluOpType.mult)
                    else:
                        stg = stgp.tile([DH, T], F32, tag="stg")
                        nc.vector.tensor_tensor(out=r(stg), in0=pv[0:DH, :],
                                                in1=rbs, op=AluOpType.mult)
                        nc.sync.dma_start(out=r(ot_tiles[cb][DH:P, :]), in_=r(stg))

                prev = None
                for h in range(H):
                    scores_exp(h)
                    if prev is not None:
                        pv_norm(prev)
                    prev = h
                pv_norm(prev)
                return ot_tiles

            def out_proj_residual(wtiles, ot_tiles, resid, tag):
                """Y0.T[dc] = Wo @ O.T + resid."""
                outs = []
                for dc in range(NDC):
                    ps = psA.tile([P, T], F32, tag="psA")
                    for ic in range(NDC):
                        nc.tensor.matmul(ps, r(wtiles[ic][:, dc * P:(dc + 1) * P]),
                                         r(ot_tiles[ic]), start=(ic == 0), stop=(ic == NDC - 1))
                    o = prelnp.tile([P, T], F32, tag="preln")
                    nc.vector.scalar_tensor_tensor(out=r(o), in0=ps, scalar=1.0,
                                                   in1=resid[dc], op0=AluOpType.mult,
                                                   op1=AluOpType.add)
                    outs.append(o)
                return outs

            def layer_norm(src, g, be, tag, round_out=True):
                """Feature-major layernorm over the partition (d) dim."""
                s1 = psB.tile([1, T], F32, tag="psB")
                s2 = psB.tile([1, T], F32, tag="psB")
                for dc in range(NDC):
                    nc.tensor.matmul(s1, r(ones128), r(src[dc]),
                                     start=(dc == 0), stop=(dc == NDC - 1))
                for dc in range(NDC):
                    sq = sqp.tile([P, T], F32, tag="sq")
                    nc.vector.tensor_tensor(out=r(sq), in0=src[dc], in1=src[dc],
                                            op=AluOpType.mult)
                    nc.tensor.matmul(s2, r(ones128), r(sq),
                                     start=(dc == 0), stop=(dc == NDC - 1))
                s1s = smp.tile([1, T], F32, tag="sm")
                nc.scalar.activation(out=r(s1s), in_=s1, func=AF.Copy)
                s2s = smp.tile([1, T], F32, tag="sm")
                nc.scalar.activation(out=r(s2s), in_=s2, func=AF.Copy)
                s1b = psA.tile([P, T], F32, tag="psA")
                nc.tensor.matmul(s1b, r(ones_row), r(s1s), start=True, stop=True)
                s2b = psA.tile([P, T], F32, tag="psA")
                nc.tensor.matmul(s2b, r(ones_row), r(s2s), start=True, stop=True)
                # mean and rstd, full-width [128, T]
                mw = lnwp.tile([P, T], F32, tag="lnw_m")
                nc.vector.tensor_scalar(out=mw, in0=s1b, scalar1=-1.0 / D, scalar2=None,
                                        op0=AluOpType.mult)  # -mu
                msq = lnwp.tile([P, T], F32, tag="lnw_v")
                nc.vector.tensor_tensor(out=msq, in0=mw, in1=mw, op=AluOpType.mult)
                nc.vector.tensor_scalar(out=msq, in0=msq, scalar1=-1.0, scalar2=None,
                                        op0=AluOpType.mult)  # -mu^2
                nc.vector.scalar_tensor_tensor(out=msq, in0=s2b, scalar=1.0 / D,
                                               in1=msq, op0=AluOpType.mult,
                                               op1=AluOpType.add)  # var
                nc.scalar.activation(out=msq, in_=msq, func=AF.Sqrt, bias=eps_t[:, 0:1])
                nc.vector.reciprocal(out=msq, in_=msq)  # rstd
                outs = []
                for dc in range(NDC):
                    o = postlnp.tile([P, T], F32, tag="postln")
                    ow = r(o) if round_out else o
                    nc.vector.tensor_tensor(out=ow, in0=src[dc], in1=mw,
                                            op=AluOpType.add)  # x - mu
                    nc.vector.tensor_tensor(out=ow, in0=o, in1=msq,
                                            op=AluOpType.mult)  # * rstd
                    nc.vector.tensor_scalar(out=(r(o) if round_out else o),
                                            in0=o, scalar1=g[:, dc:dc + 1],
                                            scalar2=be[:, dc:dc + 1],
                                            op0=AluOpType.mult, op1=AluOpType.add)
                    outs.append(o)
                return outs

            def body():
              for b in [bb for _ in range(reps) for bb in range(nb)]:
                # ---- load per-b inputs ----
                xts = []
                for dc in range(NDC):
                    t = xtp.tile([P, T], F32, tag="xt")
                    nc.sync.dma_start(out=r(t), in_=xt.ap()[b, dc * P:(dc + 1) * P, :])
                    xts.append(t)
                mts = []
                for kc in range(NDC):
                    t = mtp.tile([P, T], F32, tag="mt")
                    nc.sync.dma_start(out=t, in_=maskt.ap()[b, kc * P:(kc + 1) * P, :])
                    mts.append(t)
                ebias_t = smp.tile([P, NDC], F32, tag="sm_eb")
                nc.sync.dma_start(out=ebias_t, in_=ebias.ap()[b])

                # ---- self attention ----
                wq = load_w_tiles("wqt1", "wa")
                qt = proj_fm(wq, xts, qtp, "qt")
                wk = load_w_tiles("wkt1", "wa")
                kt = proj_fm(wk, xts, ktp, "kt")
                wv = load_w_tiles("wvt1", "wa")
                vo = proj_vones(wv, xts)
                ot_t = attention(qt, kt, vo, True, mts=mts)
                wo = load_w_tiles("wot1", "wa")
                y0 = out_proj_residual(wo, ot_t, xts, "y0")
                yt = layer_norm(y0, tgbe["g1"], tgbe["be1"], "yt")

                # ---- cross attention ----
                ets = []
                for dc in range(NDC):
                    t = etp.tile([P, T], F32, tag="et")
                    nc.sync.dma_start(out=r(t), in_=et.ap()[b, dc * P:(dc + 1) * P, :])
                    ets.append(t)
                wq2 = load_w_tiles("wqt2", "wa")
                qt2 = proj_fm(wq2, yt, qtp, "qt")
                wk2 = load_w_tiles("wkt2", "wa")
                kt2 = proj_fm(wk2, ets, ktp, "kt")
                wv2 = load_w_tiles("wvt2", "wa")
                vo2 = proj_vones(wv2, ets)
                ot2 = attention(qt2, kt2, vo2, False, ebias_t=ebias_t)
                wo2 = load_w_tiles("wot2", "wa")
                z0 = out_proj_residual(wo2, ot2, yt, "z0")
                zt = layer_norm(z0, tgbe["g2"], tgbe["be2"], "zt")

                # ---- FFN ----
                w1tiles = []
                for dc in range(NDC):
                    t = w1p.tile([P, FFN], F32, tag="w1")
                    nc.sync.dma_start(out=r(t), in_=w1t.ap()[dc * P:(dc + 1) * P, :])
                    w1tiles.append(t)
                fps = [psB.tile([P, T], F32, tag="psB", name=f"fps{i}") for i in range(NDC)]
                h_tiles = {}

                def ffn_h(hc):
                    hp = psA.tile([P, T], F32, tag="psA")
                    for dc in range(NDC):
                        nc.tensor.matmul(hp, r(w1tiles[dc][:, hc * P:(hc + 1) * P]),
                                         r(zt[dc]), start=(dc == 0), stop=(dc == NDC - 1))
                    ht = htp.tile([P, T], F32, tag="ht")
                    nc.vector.tensor_scalar(out=r(ht), in0=hp, scalar1=tb1[:, hc:hc + 1],
                                            scalar2=None, op0=AluOpType.add)
                    nc.vector.scalar_tensor_tensor(out=r(ht), in0=ht, scalar=0.01, in1=ht,
                                                   op0=AluOpType.mult, op1=AluOpType.max)
                    h_tiles[hc] = ht

                def ffn_f(hc):
                    ht = h_tiles.pop(hc)
                    w2tile = w2p.tile([P, D], F32, tag="w2")
                    nc.sync.dma_start(out=r(w2tile), in_=w2t.ap()[hc * P:(hc + 1) * P, :])
                    for dc in range(NDC):
                        nc.tensor.matmul(fps[dc], r(w2tile[:, dc * P:(dc + 1) * P]),
                                         r(ht), start=(hc == 0), stop=(hc == NHC - 1))

                prevh = None
                for hc in range(NHC):
                    ffn_h(hc)
                    if prevh is not None:
                        ffn_f(prevh)
                    prevh = hc
                ffn_f(prevh)

                out0 = []
                for dc in range(NDC):
                    o = prelnp.tile([P, T], F32, tag="preln")
                    nc.vector.scalar_tensor_tensor(out=r(o), in0=fps[dc],
                                                   scalar=tb2[:, dc:dc + 1], in1=zt[dc],
                                                   op0=AluOpType.add, op1=AluOpType.add)
                    out0.append(o)
                outt = layer_norm(out0, tgbe["g3"], tgbe["be3"], "outt", round_out=False)
                for dc in range(NDC):
                    nc.sync.dma_start(out=ot.ap()[b, dc * P:(dc + 1) * P, :],
                                      in_=outt[dc])

            if loop_n > 1:
                with tc.For_i(0, loop_n, 1):
                    body()
            else:
                body()

    nc.compile()
    _BUILD_CACHE[key] = nc
    return nc


def prep_core_inputs(inputs, nb=NB):
    """Host-side prep: transpose weights/activations, build masks, shard over cores."""
    X = np.asarray(inputs["X"], np.float32)
    E = np.asarray(inputs["enc_outputs"], np.float32)
    dv = np.asarray(inputs["dec_valid_lens"])
    ev = np.asarray(inputs["enc_valid_lens"])
    pos = np.arange(T)

    shared = {
        "w1t": np.ascontiguousarray(np.asarray(inputs["W1"], np.float32).T),
        "w2t": np.ascontiguousarray(np.asarray(inputs["W2"], np.float32).T),
        "b1c": np.ascontiguousarray(np.asarray(inputs["b1"], np.float32).reshape(NHC, P).T),
        "b2c": np.ascontiguousarray(np.asarray(inputs["b2"], np.float32).reshape(NDC, P).T),
    }
    for j in (1, 2, 3):
        shared[f"g{j}c"] = np.ascontiguousarray(
            np.asarray(inputs[f"g{j}"], np.float32).reshape(NDC, P).T)
        shared[f"be{j}c"] = np.ascontiguousarray(
            np.asarray(inputs[f"be{j}"], np.float32).reshape(NDC, P).T)
    for n, src in [("wqt1", "Wq1"), ("wkt1", "Wk1"), ("wvt1", "Wv1"), ("wot1", "Wo1"),
                   ("wqt2", "Wq2"), ("wkt2", "Wk2"), ("wvt2", "Wv2"), ("wot2", "Wo2")]:
        shared[n] = np.ascontiguousarray(np.asarray(inputs[src], np.float32).T)

    in_maps = []
    ncores = X.shape[0] // nb
    for c in range(ncores):
        sl = slice(c * nb, (c + 1) * nb)
        xt = np.ascontiguousarray(X[sl].transpose(0, 2, 1))
        et = np.ascontiguousarray(E[sl].transpose(0, 2, 1))
        # self mask: maskt[b][k, q] = NEG where k >= dec_valid[b, q]
        mk = (pos[None, :, None] >= dv[sl][:, None, :]).astype(np.float32) * NEG
        # cross bias per k: ebias[b, p, kc] for k = kc*128 + p
        eb = (pos[None, :] >= ev[sl][:, None]).astype(np.float32) * NEG
        eb = np.ascontiguousarray(eb.reshape(nb, NDC, P).transpose(0, 2, 1))
        m = {"xt": xt, "et": et, "maskt": np.ascontiguousarray(mk), "ebias": eb}
        m.update(shared)
        in_maps.append(m)
    return in_maps


def kernel(**inputs):
    from concourse import bass_utils

    nc = build(NB)
    in_maps = prep_core_inputs(inputs, NB)
    res = bass_utils.run_bass_kernel_spmd(nc, in_maps, core_ids=list(range(NCORES)))
    outs = [r["ot"].transpose(0, 2, 1) for r in res.results]  # [nb, T, D] each
    return np.ascontiguousarray(np.concatenate(outs, axis=0).astype(np.float32))
